# revision 45
# baseline (speedup 1.0000x reference)
"""Trainium2 Bass kernel for nn_Actor (dense MLP trunk + 64 softmax heads).

Data-parallel over 8 NeuronCores: batch 4096 -> 512 rows/core, weights
replicated. Feature-major trunk (activations [features, batch]) so layer
outputs feed the next contraction without transposes; heads run batch-major
so per-head softmax reduces along the free dim.

Precision: both trunk layers run fp8-e4m3 DoubleRow matmuls (256-deep
contraction per instruction at 0.5 cyc/col; weights pre-scaled x256 and
x pre-scaled x32 into e4m3's normal range, compensated via the relu
scale= parameter), heads run bf16 with each stationary h2 tile reused
across a pair of column chunks. End-to-end ||err||/||ref|| ~= 4.9e-3.

Self-contained: hardcodes shapes; host-side prep packs head weights into one
[1024, 1280] GEMM whose columns are already in the final output order
(per vehicle v: rsu[2v] | rsu[2v+1] | lay[2v] | lay[2v+1]); head bias is
folded in multiplicatively via exp(bias).
"""

import os
import numpy as np

B, IN_DIM, HIDDEN, H2 = 4096, 2048, 2048, 1024
V, R, L = 16, 32, 8
OUTC = V * (2 * R + 2 * L)          # 1280
NCORES = 8
BC = B // NCORES                    # 512 batch rows per core
KT1 = IN_DIM // 128                 # 16 k-tiles, layer 1
MT1 = HIDDEN // 128                 # 16 m-tiles, layer 1
KT2 = HIDDEN // 128                 # 16 k-tiles, layer 2
MT2 = H2 // 128                     # 8 m-tiles, layer 2
KTH = H2 // 128                     # 8 k-tiles, heads
BT = BC // 128                      # 4 batch tiles per core
CW = 320                            # head chunk width = 4 vehicles
NCH = OUTC // CW                    # 4 chunks
VC = CW // (2 * (R + L))            # 4 vehicles per chunk

_CACHE = {}
LAST_RESULTS = None                 # BassKernelResults from the last run


def _build():
    import bass_rust
    import concourse.bacc as bacc
    import concourse.mybir as mybir
    import concourse.tile as tile

    F32, F32R = mybir.dt.float32, mybir.dt.float32r
    BF16 = mybir.dt.bfloat16
    F8 = mybir.dt.float8e4
    DR = mybir.MatmulPerfMode.DoubleRow
    Relu = mybir.ActivationFunctionType.Relu
    Exp = mybir.ActivationFunctionType.Exp
    X = mybir.AxisListType.X

    nc = bacc.Bacc("TRN2", target_bir_lowering=False, debug=False,
                   num_devices=NCORES)

    xt = nc.dram_tensor("xt", [128, KT1, BC], F8, kind="ExternalInput")
    w1t = nc.dram_tensor("w1t", [MT1, 128, KT1, 128], F8, kind="ExternalInput")
    b1c = nc.dram_tensor("b1c", [128, MT1], F32, kind="ExternalInput")
    w2t = nc.dram_tensor("w2t", [MT2, 128, KT2, 128], F8, kind="ExternalInput")
    b2c = nc.dram_tensor("b2c", [128, MT2], F32, kind="ExternalInput")
    wht = nc.dram_tensor("wht", [128, KTH, OUTC], BF16, kind="ExternalInput")
    ebr = nc.dram_tensor("ebr", [128, OUTC], BF16, kind="ExternalInput")
    out = nc.dram_tensor("out", [BC, OUTC], F32, kind="ExternalOutput")

    with tile.TileContext(nc) as tc:
        with (
            tc.tile_pool(name="const", bufs=1) as cp,
            tc.tile_pool(name="wpool", bufs=8) as wp,
            tc.tile_pool(name="sm", bufs=4) as sp,
            tc.tile_pool(name="ps", bufs=4, space="PSUM") as ps,
            tc.tile_pool(name="psh", bufs=2, space="PSUM") as psh,
        ):
            # PE warmup: ~3.5us of dense dummy matmuls in the otherwise
            # DMA-bound start so the HAM clock gate opens before the first
            # real matmuls and they run at 2.4GHz.
            warm = cp.tile([128, 128], BF16, tag="warm")
            nc.gpsimd.memset(warm[:], 0.0)
            wacc = psh.tile([128, CW], F32, tag="hacc0")
            for i in range(30):
                nc.tensor.matmul(wacc[:, 0:128], warm[:], warm[:],
                                 start=True, stop=True)

            xt_sb = cp.tile([128, KT1, BC], F8, tag="xt")
            h1_sb = cp.tile([128, MT1, BC], F8, tag="h1")
            h2_sb = cp.tile([128, MT2, BC], BF16, tag="h2")
            wh_sb = cp.tile([128, KTH, OUTC], BF16, tag="wh")
            eb_sb = cp.tile([128, OUTC], BF16, tag="eb")
            b1_sb = cp.tile([128, MT1], F32, tag="b1")
            b2_sb = cp.tile([128, MT2], F32, tag="b2")

            # w1 stream on the sync ring; xt + small constants on the ACT
            # ring. The first four m-tiles run k-outer/m-inner on 4 PSUM
            # banks so the PE consumes each arriving xt chunk 4x and stays
            # dense through the DMA-bound ramp; w1[0..3] are loaded
            # k-chunk-interleaved to match.
            RM = 4                       # m-tiles in the ramp group
            rw1 = []
            for mi in range(RM):
                w1m = wp.tile([128, KT1, 128], F8, tag="w1m")
                rw1.append(w1m)
            for k0, kn in [(0, 2), (2, 2), (4, 4), (8, 4), (12, 4)]:
                for mi in range(RM):
                    nc.sync.dma_start(rw1[mi][:, k0:k0 + kn, :],
                                      w1t.ap()[mi][:, k0:k0 + kn, :])
            for k0 in range(0, KT1, 4):
                nc.scalar.dma_start(xt_sb[:, k0:k0 + 4, :],
                                    xt.ap()[:, k0:k0 + 4, :])
            nc.scalar.dma_start(b1_sb[:], b1c.ap())
            nc.scalar.dma_start(b2_sb[:], b2c.ap())

            raccs = []
            for mi in range(RM):
                racc = ps.tile([128, 512], F32, tag="acc")
                raccs.append(racc)
            for k in range(0, KT1, 2):
                for mi in range(RM):
                    nc.tensor.matmul(raccs[mi][:], rw1[mi][:, k:k + 2, :],
                                     xt_sb[:, k:k + 2, :],
                                     start=(k == 0), stop=(k == KT1 - 2),
                                     perf_mode=DR)
            for mi in range(RM):
                nc.scalar.activation(h1_sb[:, mi, :], raccs[mi][:], Relu,
                                     bias=b1_sb[:, mi:mi + 1], scale=1.0 / 512.0)

            # Layer 1 remainder: h1[m] = relu(sum_k w1[k,m].T @ xt[k] + b1[m])
            for m in range(RM, MT1):
                w1m = wp.tile([128, KT1, 128], F8, tag="w1m")
                nc.sync.dma_start(w1m[:], w1t.ap()[m])
                acc = ps.tile([128, 512], F32, tag="acc")
                for k in range(0, KT1, 2):
                    nc.tensor.matmul(acc[:], w1m[:, k:k + 2, :],
                                     xt_sb[:, k:k + 2, :],
                                     start=(k == 0), stop=(k == KT1 - 2),
                                     perf_mode=DR)
                relu = nc.scalar.activation(h1_sb[:, m, :], acc[:], Relu,
                                            bias=b1_sb[:, m:m + 1],
                                            scale=1.0 / 512.0)
                # wh prefetch paced to L1 progress so it can't starve the
                # xt/w1 streams of HBM bandwidth at kernel start
                if m >= 4 and m % 2 == 0:
                    kh = (m - 4) // 2
                    whd = nc.gpsimd.dma_start(wh_sb[:, kh, :],
                                              wht.ap()[:, kh, :])
                    bass_rust.add_dep_helper(whd.ins, relu.ins, sync=True,
                                             reason="pace wh prefetch")
                if m == 15:
                    whd = nc.gpsimd.dma_start(wh_sb[:, KTH - 2, :],
                                              wht.ap()[:, KTH - 2, :])
                    bass_rust.add_dep_helper(whd.ins, relu.ins, sync=True,
                                             reason="pace wh prefetch")
                    ebd = nc.gpsimd.dma_start(eb_sb[:], ebr.ap())
                    bass_rust.add_dep_helper(ebd.ins, relu.ins, sync=True,
                                             reason="pace eb prefetch")

            # Layer 2: h2[m] = relu(sum_k w2[k,m].T @ h1[k] + b2[m])
            for m in range(MT2):
                w2m = wp.tile([128, KT2, 128], F8, tag="w2m")
                nc.sync.dma_start(w2m[:], w2t.ap()[m])
                acc = ps.tile([128, 512], F32, tag="acc")
                for k in range(0, KT2, 2):
                    nc.tensor.matmul(acc[:], w2m[:, k:k + 2, :],
                                     h1_sb[:, k:k + 2, :],
                                     start=(k == 0), stop=(k == KT2 - 2),
                                     perf_mode=DR)
                relu = nc.scalar.activation(h2_sb[:, m, :], acc[:], Relu,
                                            bias=b2_sb[:, m:m + 1],
                                            scale=1.0 / 4096.0)
                if m == 0:
                    whd = nc.gpsimd.dma_start(wh_sb[:, KTH - 1, :],
                                              wht.ap()[:, KTH - 1, :])
                    bass_rust.add_dep_helper(whd.ins, relu.ins, sync=True,
                                             reason="pace wh prefetch")

            # Heads: logits[b, :] = h2[:, b].T @ wh, f32r, k-outer so each
            # stationary h2 tile is loaded once and reused for all 4 chunks.
            # softmax(l + bias) == exp(l)*exp(bias) / sum(exp(l)*exp(bias))
            for bt in range(BT):
                bsl = slice(bt * 128, (bt + 1) * 128)
                for pr in range(NCH // 2):
                    accs = []
                    for ci in range(2):
                        hacc = psh.tile([128, CW], F32, tag=f"hacc{ci}")
                        accs.append(hacc)
                    for k in range(KTH):
                        for ci in range(2):
                            c = 2 * pr + ci
                            nc.tensor.matmul(accs[ci][:], h2_sb[:, k, bsl],
                                             wh_sb[:, k, c * CW:(c + 1) * CW],
                                             start=(k == 0), stop=(k == KTH - 1))
                    # pair-level softmax on [128, 640] so per-op overhead
                    # is paid once per pair; divisor muls on GpSimd so DVE
                    # (ebmul+reduces+recip) stays below the PE phase time.
                    # The very last pair instead runs two per-chunk chains on
                    # DVE for the shortest post-matmul critical path.
                    last = (bt == BT - 1 and pr == NCH // 2 - 1)
                    if not last:
                        widths = [(2 * pr * CW, 2 * CW, 2 * VC, [0, 1])]
                    else:
                        widths = [(2 * pr * CW, CW, VC, [0]),
                                  ((2 * pr + 1) * CW, CW, VC, [1])]
                    for c0, PW, VP, cis in widths:
                        et = sp.tile([128, 2 * CW], BF16, tag="et")
                        for ci in cis:
                            nc.scalar.activation(
                                et[:, (ci - cis[0]) * CW:(ci - cis[0] + 1) * CW],
                                accs[ci][:], Exp)
                        etv = et[:, 0:PW]
                        num = sp.tile([128, 2 * CW], BF16, tag="num")
                        nc.vector.tensor_mul(num[:, 0:PW], etv,
                                             eb_sb[:, c0:c0 + PW])

                        nv = num[:, 0:PW].rearrange("p (v x) -> p v x", v=VP)
                        rsu4 = nv[:, :, 0:2 * R].rearrange(
                            "p v (h c) -> p v h c", h=2)
                        lay4 = nv[:, :, 2 * R:].rearrange(
                            "p v (h c) -> p v h c", h=2)
                        sums = sp.tile([128, 8 * VC], F32, tag="sums")
                        s_r = sums[:, 0:2 * VP].rearrange("p (v h) -> p v h", h=2)
                        s_l = sums[:, 2 * VP:4 * VP].rearrange(
                            "p (v h) -> p v h", h=2)
                        nc.vector.reduce_sum(out=s_r.unsqueeze(3), in_=rsu4, axis=X)
                        nc.vector.reduce_sum(out=s_l.unsqueeze(3), in_=lay4, axis=X)
                        rec = sp.tile([128, 8 * VC], F32, tag="rec")
                        nc.vector.reciprocal(rec[:, 0:4 * VP], sums[:, 0:4 * VP])

                        o_sb = sp.tile([128, 2 * CW], F32, tag="o")
                        ov = o_sb[:, 0:PW].rearrange("p (v x) -> p v x", v=VP)
                        orsu = ov[:, :, 0:2 * R].rearrange(
                            "p v (h c) -> p v h c", h=2)
                        olay = ov[:, :, 2 * R:].rearrange(
                            "p v (h c) -> p v h c", h=2)
                        r_r = rec[:, 0:2 * VP].rearrange("p (v h) -> p v h", h=2)
                        r_l = rec[:, 2 * VP:4 * VP].rearrange(
                            "p (v h) -> p v h", h=2)
                        meng = nc.vector if last else nc.gpsimd
                        meng.tensor_mul(
                            orsu, rsu4,
                            r_r.unsqueeze(3).broadcast_to([128, VP, 2, R]))
                        meng.tensor_mul(
                            olay, lay4,
                            r_l.unsqueeze(3).broadcast_to([128, VP, 2, L]))
                        nc.sync.dma_start(out.ap()[bsl, c0:c0 + PW],
                                          o_sb[:, 0:PW])

    nc.compile()
    return nc


def _prep_shared(w1, b1, w2, b2, w_rsu, b_rsu, w_lay, b_lay):
    import ml_dtypes
    f, bf = np.float32, ml_dtypes.bfloat16
    f8 = ml_dtypes.float8_e4m3
    w1t = np.ascontiguousarray(
        np.clip(w1 * 256.0, -240, 240).astype(f8)
        .reshape(KT1, 128, MT1, 128).transpose(2, 1, 0, 3))
    w2t = np.ascontiguousarray(
        np.clip(w2 * 256.0, -240, 240).astype(f8)
        .reshape(KT2, 128, MT2, 128).transpose(2, 1, 0, 3))
    b1c = np.ascontiguousarray(16.0 * b1.reshape(MT1, 128).T, dtype=f)
    b2c = np.ascontiguousarray(b2.reshape(MT2, 128).T, dtype=f)

    wh = np.empty((H2, OUTC), dtype=f)
    bh = np.empty((OUTC,), dtype=f)
    for v in range(V):
        c = 2 * (R + L) * v
        wh[:, c:c + R] = w_rsu[2 * v]
        wh[:, c + R:c + 2 * R] = w_rsu[2 * v + 1]
        wh[:, c + 2 * R:c + 2 * R + L] = w_lay[2 * v]
        wh[:, c + 2 * R + L:c + 2 * (R + L)] = w_lay[2 * v + 1]
        bh[c:c + R] = b_rsu[2 * v]
        bh[c + R:c + 2 * R] = b_rsu[2 * v + 1]
        bh[c + 2 * R:c + 2 * R + L] = b_lay[2 * v]
        bh[c + 2 * R + L:c + 2 * (R + L)] = b_lay[2 * v + 1]
    wht = np.ascontiguousarray(
        wh.astype(bf).reshape(KTH, 128, OUTC).transpose(1, 0, 2))
    ebr = np.ascontiguousarray(
        np.broadcast_to(np.exp(bh).astype(bf)[None, :], (128, OUTC)))
    return {"w1t": w1t, "b1c": b1c, "w2t": w2t, "b2c": b2c,
            "wht": wht, "ebr": ebr}


def kernel(x, w1, b1, w2, b2, w_rsu, b_rsu, w_lay, b_lay):
    global LAST_RESULTS
    import ml_dtypes
    from concourse.bass_utils import run_bass_kernel_spmd

    if "nc" not in _CACHE:
        _CACHE["nc"] = _build()
    nc = _CACHE["nc"]

    shared = _prep_shared(np.asarray(w1, np.float32), np.asarray(b1, np.float32),
                          np.asarray(w2, np.float32), np.asarray(b2, np.float32),
                          np.asarray(w_rsu, np.float32), np.asarray(b_rsu, np.float32),
                          np.asarray(w_lay, np.float32), np.asarray(b_lay, np.float32))

    # x [B, IN] -> per-core xt [128, KT1, BC] with [p, k, n] = x[core*BC+n, k*128+p]
    # fp8 e4m3 with x*16 so small values clear the subnormal range; the
    # combined 16*256 scale comes out in the L1 relu (scale=1/4096)
    xt_full = np.clip(np.ascontiguousarray(np.asarray(x, np.float32).T) * 32.0,
                      -240, 240) \
        .astype(ml_dtypes.float8_e4m3).reshape(KT1, 128, B).transpose(1, 0, 2)
    in_maps = []
    for c in range(NCORES):
        m = dict(shared)
        m["xt"] = np.ascontiguousarray(xt_full[:, :, c * BC:(c + 1) * BC])
        in_maps.append(m)

    trace = os.environ.get("KERNEL_TRACE", "") == "1"
    LAST_RESULTS = run_bass_kernel_spmd(nc, in_maps, core_ids=list(range(NCORES)),
                                        trace=trace)
    return np.concatenate([r["out"] for r in LAST_RESULTS.results], axis=0)
